# revision 6
# baseline (speedup 1.0000x reference)
"""Trainium2 Bass kernel for ContextQueryAttention (BiDAF-style trilinear attention).

Math (per batch b):
  S[n,m] = ctx[n]·w_c + q[m]·w_q + (ctx[n]*w_m)·q[m]
  A  = softmax_m(S + qmask_bias) ; Bm = softmax_n(S + cmask_bias)
  c2q = A @ q ;  q2c = A @ Bm^T @ ctx
  out = concat([ctx, c2q, ctx*c2q, ctx*q2c], -1)

Device decomposition (per core, 4 batches), all heavy matmuls fp8e4 DoubleRow
(2 k-tiles per instruction, 0.5 cyc/row):
  S64 = ctxT8.T @ (64*wm*q)8  +  onehot-pair trick adding 64*cwc[n] (a+residual fp8 rows)
  E8[n,m]   = fp8(exp(S64/64))              (Act, PSUM->SBUF)
  colsum[m] = czero-cols of C1 matmul ;  C1s8 = fp8(C1raw * expqb4/colsum)
  ET8       = PE-transpose of E8 (fp8, psum elem-step-2) -> SBUF
  c2q_raw   = ET8.T @ qs8       (qs8 = fp8(q*expqb4), cols 512:514 = fp8(expqb4) -> rowsums)
  q2c_raw   = ET8.T @ C1s8
  out8      = fp8([c2q_raw | q2c_raw]) ; rows16 = f16(rowsums)
Host: expqb4 = exp(q@w_q + qmask)/4, cwc = ctx@w_c, fp8 packing; afterwards
  c2q = c2q_raw/rows, q2c = q2c_raw/rows, out = concat([ctx, c2q, ctx*c2q, ctx*q2c]).
The exact softmax shifts cancel: A = E*expqb4/rows row-wise; masked m have
expqb4 == 0 exactly, masked n are zeroed in ctxm8 (czero).
"""

import numpy as np
import ml_dtypes

F8 = ml_dtypes.float8_e4m3fn

B, N, M, D = 32, 1024, 256, 512
NCORES = 8
BL = B // NCORES          # batches per core
NT = N // 128             # 8 context row tiles
MT = M // 128             # 2 query row tiles
DC = D // 128             # 4 feature chunks
SC = 64.0                 # fp8 scale for the trilinear weights / cwc rows

_built = {}


def _build_nc(repeat=1):
    import concourse.bass as bass  # noqa: F401
    import concourse.mybir as mybir
    import concourse.tile as tile
    from concourse import bacc
    from concourse.masks import make_identity

    f32 = mybir.dt.float32
    f16 = mybir.dt.float16
    f8 = mybir.dt.float8e4
    EXP = mybir.ActivationFunctionType.Exp
    MUL = mybir.AluOpType.mult
    DR = mybir.MatmulPerfMode.DoubleRow

    nc = bacc.Bacc("TRN2", target_bir_lowering=False, debug=False)
    cm8_d = nc.dram_tensor("cm8", (BL, 128, NT * 516), f8, kind="ExternalInput")
    ct8_d = nc.dram_tensor("ct8", (BL, 128, DC * 1024), f8, kind="ExternalInput")
    qw8_d = nc.dram_tensor("qw8", (BL, 128, DC * 256), f8, kind="ExternalInput")
    qs8_d = nc.dram_tensor("qs8", (BL, 128, MT * 516), f8, kind="ExternalInput")
    cw8_d = nc.dram_tensor("cw8", (2, 2, BL * 1024), f8, kind="ExternalInput")
    aux_d = nc.dram_tensor("aux", (128, BL * MT), f32, kind="ExternalInput")
    out_d = nc.dram_tensor("out", (BL, 128, NT * 1024), f8, kind="ExternalOutput")
    rws_d = nc.dram_tensor("rws", (BL, 128, 16), f16, kind="ExternalOutput")

    # out-drain engine split: True -> Act(scalar), False -> DVE(vector)
    ACT_NT = (True, False, True, False, True, False, True, False)

    with tile.TileContext(nc) as tc:
        with (
            tc.tile_pool(name="singles", bufs=1) as singles,
            tc.tile_pool(name="p_cm", bufs=2) as p_cm,
            tc.tile_pool(name="p_ct", bufs=2) as p_ct,
            tc.tile_pool(name="p_qw", bufs=2) as p_qw,
            tc.tile_pool(name="p_qs", bufs=2) as p_qs,
            tc.tile_pool(name="p_e", bufs=2) as p_e,
            tc.tile_pool(name="p_et", bufs=2) as p_et,
            tc.tile_pool(name="p_c1", bufs=2) as p_c1,
            tc.tile_pool(name="p_out", bufs=2) as p_out,
            tc.tile_pool(name="p_sm", bufs=2) as p_sm,
            tc.tile_pool(name="ps_a", bufs=2, space="PSUM") as ps_a,
            tc.tile_pool(name="ps_et", bufs=1, space="PSUM") as ps_et,
            tc.tile_pool(name="ps_o", bufs=2, space="PSUM") as ps_o,
            tc.tile_pool(name="ps_sm", bufs=1, space="PSUM") as ps_sm,
        ):
            # one-time constants (tiny DMAs)
            aux_sb = singles.tile([128, BL * MT], f32)
            cw8_sb = singles.tile([2, 2, BL * 1024], f8)
            id32 = singles.tile([128, 128], f32)
            make_identity(nc, id32)
            id8 = singles.tile([128, 128], f8)
            nc.vector.tensor_copy(id8, id32)
            # one-hot rhs for the cwc bias matmul: rows (p=0,k=0),(p=1,k=0) = 1
            rex = singles.tile([2, 2, 256], f8)
            nc.vector.memset(rex, 0.0)
            nc.vector.memset(rex[0:2, 0, :], 1.0)

            n_iters = repeat * BL

            def in_phase(it):
                b = it % BL
                ct = p_ct.tile([128, DC, 1024], f8, tag="ct")
                nc.sync.dma_start(
                    ct, ct8_d.ap()[b].rearrange("p (a c) -> p a c", c=1024)
                )
                qw = p_qw.tile([128, DC, 256], f8, tag="qw")
                nc.sync.dma_start(
                    qw, qw8_d.ap()[b].rearrange("p (a c) -> p a c", c=256)
                )
                if it == 0:
                    nc.sync.dma_start(cw8_sb, cw8_d.ap())
                    nc.sync.dma_start(aux_sb, aux_d.ap())
                cm = p_cm.tile([128, NT, 516], f8, tag="cm")
                nc.sync.dma_start(
                    cm, cm8_d.ap()[b].rearrange("p (a c) -> p a c", c=516)
                )
                qs = p_qs.tile([128, MT, 516], f8, tag="qs")
                nc.sync.dma_start(
                    qs, qs8_d.ap()[b].rearrange("p (a c) -> p a c", c=516)
                )
                return ct, qw, cm, qs

            def s_phase(it, ct, qw):
                b = it % BL
                # S (fp8 DR, cwc added via 2-partition one-hot matmul),
                # E = exp(S/64) on Act, two context tiles per psum bank.
                E8 = p_e.tile([128, NT, 256], f8, tag="E8")
                for pp in range(NT // 2):
                    s_ps = ps_a.tile([128, 512], f32, tag="a")
                    for j in range(2):
                        nt = 2 * pp + j
                        o = s_ps[:, j * 256:(j + 1) * 256]
                        for dp in range(DC // 2):
                            nc.tensor.matmul(
                                o,
                                ct[:, 2 * dp:2 * dp + 2, nt * 128:(nt + 1) * 128],
                                qw[:, 2 * dp:2 * dp + 2, :],
                                start=(dp == 0), stop=False, perf_mode=DR,
                            )
                        nc.tensor.matmul(
                            o,
                            cw8_sb[:, :, b * 1024 + nt * 128:b * 1024 + (nt + 1) * 128],
                            rex,
                            start=False, stop=True, perf_mode=DR,
                        )
                    nc.scalar.activation(
                        E8[:, 2 * pp:2 * pp + 2, :], s_ps, EXP,
                        bias=0.0, scale=1.0 / SC,
                    )
                return E8

            def etc1_phase(it, cm, E8):
                b = it % BL
                # small psum: rows (cols 0:16), colsum (cols 16:20)
                sm_ps = ps_sm.tile([128, 20], f32, tag="sm")
                rc = p_sm.tile([128, MT], f32, tag="rc")
                rr = p_sm.tile([128, MT], f32, tag="rr")

                ET8 = p_et.tile([128, MT, 1024], f8, tag="ET8")
                C1s8 = p_c1.tile([128, MT, 512], f8, tag="C1s8")
                for mt in range(MT):
                    et_ps = ps_et.tile([128, 1024, 2], f8, tag="et")
                    for nt in range(NT):
                        nc.tensor.transpose(
                            et_ps[:, nt * 128:(nt + 1) * 128, 0],
                            E8[:, nt, mt * 128:(mt + 1) * 128],
                            id8,
                        )
                    nc.scalar.copy(ET8[:, mt, 0:512], et_ps[:, 0:512, 0])
                    nc.vector.tensor_copy(ET8[:, mt, 512:1024], et_ps[:, 512:1024, 0])

                    c1_ps = ps_a.tile([128, 512], f32, tag="a")
                    for np_ in range(NT // 2):
                        nc.tensor.matmul(
                            c1_ps,
                            E8[:, 2 * np_:2 * np_ + 2, mt * 128:(mt + 1) * 128],
                            cm[:, 2 * np_:2 * np_ + 2, 0:512],
                            start=(np_ == 0), stop=(np_ == NT // 2 - 1),
                            perf_mode=DR,
                        )
                        nc.tensor.matmul(
                            sm_ps[:, 16 + 2 * mt:18 + 2 * mt],
                            E8[:, 2 * np_:2 * np_ + 2, mt * 128:(mt + 1) * 128],
                            cm[:, 2 * np_:2 * np_ + 2, 512:514],
                            start=(np_ == 0), stop=(np_ == NT // 2 - 1),
                            perf_mode=DR,
                        )
                    nc.vector.reciprocal(
                        rc[:, mt:mt + 1], sm_ps[:, 16 + 2 * mt:17 + 2 * mt]
                    )
                    nc.scalar.mul(
                        rr[:, mt:mt + 1], rc[:, mt:mt + 1],
                        aux_sb[:, b * MT + mt:b * MT + mt + 1],
                    )
                    nc.vector.tensor_scalar(
                        C1s8[:, mt, :], c1_ps, rr[:, mt:mt + 1], None, MUL,
                    )
                return sm_ps, ET8, C1s8

            def out_phase(it, sm_ps, ET8, C1s8, qs, last=False):
                b = it % BL
                # rows first: frees sm_ps ring early for the next batch
                for nt in range(NT):
                    nc.tensor.matmul(
                        sm_ps[:, 2 * nt:2 * nt + 2],
                        ET8[:, :, nt * 128:(nt + 1) * 128],
                        qs[:, :, 512:514],
                        start=True, stop=True, perf_mode=DR,
                    )
                rows16 = p_sm.tile([128, 16], f16, tag="rows")
                nc.vector.tensor_copy(rows16, sm_ps[:, 0:16])
                nc.gpsimd.dma_start(rws_d.ap()[b], rows16)

                out_sb = p_out.tile([128, NT, 1024], f8, tag="out")
                for nt in range(NT):
                    o_ps = ps_o.tile([128, 1024], f32, tag="o")
                    lhsT = ET8[:, :, nt * 128:(nt + 1) * 128]
                    nc.tensor.matmul(
                        o_ps[:, 0:512], lhsT, qs[:, :, 0:512],
                        start=True, stop=True, perf_mode=DR,
                    )
                    nc.tensor.matmul(
                        o_ps[:, 512:1024], lhsT, C1s8[:, :, :],
                        start=True, stop=True, perf_mode=DR,
                    )
                    if last:
                        # split fine across both engines to shrink the tail
                        nc.scalar.copy(out_sb[:, nt, 0:512], o_ps[:, 0:512])
                        nc.vector.tensor_copy(out_sb[:, nt, 512:1024], o_ps[:, 512:1024])
                        nc.gpsimd.dma_start(
                            out_d.ap()[b][:, nt * 1024:(nt + 1) * 1024],
                            out_sb[:, nt, :],
                        )
                    else:
                        if ACT_NT[nt]:
                            nc.scalar.copy(out_sb[:, nt, :], o_ps)
                        else:
                            nc.vector.tensor_copy(out_sb[:, nt, :], o_ps)
                        if nt % 2 == 1:
                            nc.gpsimd.dma_start(
                                out_d.ap()[b][:, (nt - 1) * 1024:(nt + 1) * 1024],
                                out_sb[:, nt - 1:nt + 1, :].rearrange("p a c -> p (a c)"),
                            )

            pend = None
            for it in range(n_iters):
                ct, qw, cm, qs = in_phase(it)
                E8 = s_phase(it, ct, qw)
                if pend is not None:
                    out_phase(*pend)
                pend = (it, *etc1_phase(it, cm, E8), qs)
            out_phase(*pend, last=True)

    nc.compile()
    return nc


def get_nc(repeat=1):
    key = ("nc", repeat)
    if key not in _built:
        _built[key] = _build_nc(repeat)
    return _built[key]


def _f8(x):
    return np.ascontiguousarray(x.astype(F8).view(np.uint8))


def _host_prep(context, query, c_mask, q_mask, w):
    context = np.asarray(context, dtype=np.float32)
    query = np.asarray(query, dtype=np.float32)
    c_mask = np.asarray(c_mask)
    q_mask = np.asarray(q_mask)
    w = np.asarray(w, dtype=np.float32).reshape(3 * D)
    wq, wc, wm = w[:D], w[D:2 * D], w[2 * D:]

    czero = c_mask.astype(np.float32)                       # [B, N]
    cwc = context @ wc                                      # [B, N]
    qwq = query @ wq                                        # [B, M]
    expqb4 = np.where(q_mask, np.exp(qwq), 0.0).astype(np.float32) * 0.25

    # [B, N, D] -> [B, 128, NT, D] with n = nt*128 + p
    ctx_p = context.reshape(B, NT, 128, D).transpose(0, 2, 1, 3)
    czero_p = czero.reshape(B, NT, 128).transpose(0, 2, 1)  # [B, 128, NT]
    cwc_p = cwc.reshape(B, NT, 128).transpose(0, 2, 1)

    cm8 = np.zeros((B, 128, NT, 516), dtype=np.uint8)
    cm8[..., 0:512] = _f8(ctx_p * czero_p[..., None])
    cm8[..., 512:514] = _f8(czero_p)[..., None]

    # ctx^T: [B, D, N] -> [B, 128, DC, N] with d = dc*128 + p
    ctxT = context.transpose(0, 2, 1).reshape(B, DC, 128, N).transpose(0, 2, 1, 3)
    ct8 = _f8(ctxT)

    qTwm = (query * (wm * SC)[None, None, :]).transpose(0, 2, 1)
    qw8 = _f8(qTwm.reshape(B, DC, 128, M).transpose(0, 2, 1, 3))

    q_p = query.reshape(B, MT, 128, D).transpose(0, 2, 1, 3)
    eq_p = expqb4.reshape(B, MT, 128).transpose(0, 2, 1)    # [B, 128, MT]
    qs8 = np.zeros((B, 128, MT, 516), dtype=np.uint8)
    qs8[..., 0:512] = _f8(q_p * eq_p[..., None])
    qs8[..., 512:514] = _f8(eq_p)[..., None]

    # cwc a+residual rows: [2, 2, BL*1024] per core, (p, k, b*1024 + n)
    a = (SC * cwc).astype(F8)
    r = (SC * cwc - a.astype(np.float32)).astype(F8)

    in_maps = []
    for c in range(NCORES):
        bs = slice(c * BL, (c + 1) * BL)
        cw8 = np.zeros((2, 2, BL * 1024), dtype=np.uint8)
        cw8[0, 0] = a[bs].reshape(BL * N).view(np.uint8)
        cw8[1, 0] = r[bs].reshape(BL * N).view(np.uint8)
        aux = np.ascontiguousarray(
            eq_p[bs].transpose(1, 0, 2).reshape(128, BL * MT)
        )
        in_maps.append({
            "cm8": np.ascontiguousarray(cm8[bs].reshape(BL, 128, NT * 516)),
            "ct8": np.ascontiguousarray(ct8[bs].reshape(BL, 128, DC * 1024)),
            "qw8": np.ascontiguousarray(qw8[bs].reshape(BL, 128, DC * 256)),
            "qs8": np.ascontiguousarray(qs8[bs].reshape(BL, 128, MT * 516)),
            "cw8": cw8,
            "aux": aux,
        })
    return in_maps


def run_on_device(in_maps, trace=False, repeat=1, **kw):
    from concourse.bass_utils import run_bass_kernel_spmd

    nc = get_nc(repeat)
    return run_bass_kernel_spmd(
        nc, in_maps, core_ids=list(range(NCORES)), trace=trace, **kw
    )


def _assemble(context, results):
    context = np.asarray(context, dtype=np.float32)
    outs, rows = [], []
    for r in results:
        o = np.asarray(r["out"])
        if o.dtype != F8:
            o = o.view(F8)
        outs.append(o.reshape(BL, 128, NT, 1024))
        rows.append(np.asarray(r["rws"]).reshape(BL, 128, 16))
    # [B, 128, NT, 1024] -> [B, N, 1024]
    o = np.concatenate(outs, 0).astype(np.float32)
    o = o.transpose(0, 2, 1, 3).reshape(B, N, 1024)
    rw = np.concatenate(rows, 0).astype(np.float32)[:, :, 0:16:2]
    rw = rw.transpose(0, 2, 1).reshape(B, N)
    inv = 1.0 / rw
    c2q = o[:, :, 0:512] * inv[:, :, None]
    q2c = o[:, :, 512:1024] * inv[:, :, None]
    return np.concatenate(
        [context, c2q, context * c2q, context * q2c], axis=-1
    ).astype(np.float32, copy=False)


def kernel(context, query, c_mask, q_mask, w):
    in_maps = _host_prep(context, query, c_mask, q_mask, w)
    res = run_on_device(in_maps)
    return _assemble(context, res.results)


# revision 8
# speedup vs baseline: 1.0307x; 1.0307x over previous
"""Trainium2 Bass kernel for ContextQueryAttention (BiDAF-style trilinear attention).

Math (per batch b):
  S[n,m] = ctx[n]·w_c + q[m]·w_q + (ctx[n]*w_m)·q[m]
  A  = softmax_m(S + qmask_bias) ; Bm = softmax_n(S + cmask_bias)
  c2q = A @ q ;  q2c = A @ Bm^T @ ctx
  out = concat([ctx, c2q, ctx*c2q, ctx*q2c], -1)

Device decomposition (per core, 4 batches), all heavy matmuls fp8e4 DoubleRow
(2 k-tiles per instruction, 0.5 cyc/row):
  S64 = ctxT8.T @ (64*wm*q)8  +  onehot-pair trick adding 64*cwc[n] (a+residual fp8 rows)
  E8[n,m]   = fp8(exp(S64/64))              (Act, PSUM->SBUF)
  colsum[m] = czero-cols of C1 matmul ;  C1s8 = fp8(C1raw * expqb4/colsum)
  ET8       = PE-transpose of E8 (fp8, psum elem-step-2) -> SBUF
  c2q_raw   = ET8.T @ qs8       (qs8 = fp8(q*expqb4), cols 512:514 = fp8(expqb4) -> rowsums)
  q2c_raw   = ET8.T @ C1s8
  out8      = fp8([c2q_raw | q2c_raw]) ; rows16 = f16(rowsums)
Host: expqb4 = exp(q@w_q + qmask)/4, cwc = ctx@w_c, fp8 packing; afterwards
  c2q = c2q_raw/rows, q2c = q2c_raw/rows, out = concat([ctx, c2q, ctx*c2q, ctx*q2c]).
The exact softmax shifts cancel: A = E*expqb4/rows row-wise; masked m have
expqb4 == 0 exactly, masked n are zeroed in ctxm8 (czero).
"""

import numpy as np
import ml_dtypes

F8 = ml_dtypes.float8_e4m3fn

B, N, M, D = 32, 1024, 256, 512
NCORES = 8
BL = B // NCORES          # batches per core
NT = N // 128             # 8 context row tiles
MT = M // 128             # 2 query row tiles
DC = D // 128             # 4 feature chunks
SC = 64.0                 # fp8 scale for the trilinear weights / cwc rows

_built = {}


def _build_nc(repeat=1):
    import concourse.bass as bass  # noqa: F401
    import concourse.mybir as mybir
    import concourse.tile as tile
    from concourse import bacc
    from concourse.masks import make_identity

    f32 = mybir.dt.float32
    f16 = mybir.dt.float16
    f8 = mybir.dt.float8e4
    EXP = mybir.ActivationFunctionType.Exp
    MUL = mybir.AluOpType.mult
    DR = mybir.MatmulPerfMode.DoubleRow

    nc = bacc.Bacc("TRN2", target_bir_lowering=False, debug=False)
    cm8_d = nc.dram_tensor("cm8", (BL, 128, NT * 516), f8, kind="ExternalInput")
    ct8_d = nc.dram_tensor("ct8", (BL, 128, DC * 1024), f8, kind="ExternalInput")
    qw8_d = nc.dram_tensor("qw8", (BL, 128, DC * 256), f8, kind="ExternalInput")
    qs8_d = nc.dram_tensor("qs8", (BL, 128, MT * 516), f8, kind="ExternalInput")
    cw8_d = nc.dram_tensor("cw8", (2, 2, BL * 1024), f8, kind="ExternalInput")
    aux_d = nc.dram_tensor("aux", (128, BL * MT), f32, kind="ExternalInput")
    out_d = nc.dram_tensor("out", (BL, 128, NT * 1024), f8, kind="ExternalOutput")
    rws_d = nc.dram_tensor("rws", (BL, 128, 16), f16, kind="ExternalOutput")

    # out-drain engine split: True -> Act(scalar), False -> DVE(vector)
    ACT_NT = (True, False, True, False, True, False, True, False)

    with tile.TileContext(nc) as tc:
        with (
            tc.tile_pool(name="singles", bufs=1) as singles,
            tc.tile_pool(name="p_cm", bufs=2) as p_cm,
            tc.tile_pool(name="p_ct", bufs=2) as p_ct,
            tc.tile_pool(name="p_qw", bufs=2) as p_qw,
            tc.tile_pool(name="p_qs", bufs=2) as p_qs,
            tc.tile_pool(name="p_e", bufs=2) as p_e,
            tc.tile_pool(name="p_et", bufs=2) as p_et,
            tc.tile_pool(name="p_c1", bufs=2) as p_c1,
            tc.tile_pool(name="p_out", bufs=2) as p_out,
            tc.tile_pool(name="p_sm", bufs=2) as p_sm,
            tc.tile_pool(name="ps_a", bufs=2, space="PSUM") as ps_a,
            tc.tile_pool(name="ps_et", bufs=1, space="PSUM") as ps_et,
            tc.tile_pool(name="ps_o", bufs=2, space="PSUM") as ps_o,
            tc.tile_pool(name="ps_sm", bufs=1, space="PSUM") as ps_sm,
        ):
            # one-time constants (tiny DMAs)
            aux_sb = singles.tile([128, BL * MT], f32)
            cw8_sb = singles.tile([2, 2, BL * 1024], f8)
            id32 = singles.tile([128, 128], f32)
            make_identity(nc, id32)
            id8 = singles.tile([128, 128], f8)
            nc.vector.tensor_copy(id8, id32)
            # one-hot rhs for the cwc bias matmul: rows (p=0,k=0),(p=1,k=0) = 1
            rex = singles.tile([2, 2, 256], f8)
            nc.vector.memset(rex, 0.0)
            nc.vector.memset(rex[0:2, 0, :], 1.0)

            n_iters = repeat * BL

            def in_phase(it):
                b = it % BL
                ct = p_ct.tile([128, DC, 1024], f8, tag="ct")
                nc.sync.dma_start(
                    ct, ct8_d.ap()[b].rearrange("p (a c) -> p a c", c=1024)
                )
                qw = p_qw.tile([128, DC, 256], f8, tag="qw")
                nc.sync.dma_start(
                    qw, qw8_d.ap()[b].rearrange("p (a c) -> p a c", c=256)
                )
                if it == 0:
                    nc.sync.dma_start(cw8_sb, cw8_d.ap())
                    nc.sync.dma_start(aux_sb, aux_d.ap())
                cm = p_cm.tile([128, NT, 516], f8, tag="cm")
                nc.sync.dma_start(
                    cm, cm8_d.ap()[b].rearrange("p (a c) -> p a c", c=516)
                )
                qs = p_qs.tile([128, MT, 516], f8, tag="qs")
                nc.sync.dma_start(
                    qs, qs8_d.ap()[b].rearrange("p (a c) -> p a c", c=516)
                )
                return ct, qw, cm, qs

            def s_phase(it, ct, qw):
                b = it % BL
                # S (fp8 DR, cwc added via 2-partition one-hot matmul),
                # E = exp(S/64) on Act, two context tiles per psum bank.
                E8 = p_e.tile([128, NT, 256], f8, tag="E8")
                for pp in range(NT // 2):
                    s_ps = ps_a.tile([128, 512], f32, tag="a")
                    for j in range(2):
                        nt = 2 * pp + j
                        o = s_ps[:, j * 256:(j + 1) * 256]
                        for dp in range(DC // 2):
                            nc.tensor.matmul(
                                o,
                                ct[:, 2 * dp:2 * dp + 2, nt * 128:(nt + 1) * 128],
                                qw[:, 2 * dp:2 * dp + 2, :],
                                start=(dp == 0), stop=False, perf_mode=DR,
                            )
                        nc.tensor.matmul(
                            o,
                            cw8_sb[:, :, b * 1024 + nt * 128:b * 1024 + (nt + 1) * 128],
                            rex,
                            start=False, stop=True, perf_mode=DR,
                        )
                    nc.scalar.activation(
                        E8[:, 2 * pp:2 * pp + 2, :], s_ps, EXP,
                        bias=0.0, scale=1.0 / SC,
                    )
                return E8

            def etc1_phase(it, cm, E8):
                b = it % BL
                # small psum: rows (cols 0:16), colsum (cols 16:20)
                sm_ps = ps_sm.tile([128, 20], f32, tag="sm")
                rc = p_sm.tile([128, MT], f32, tag="rc")
                rr = p_sm.tile([128, MT], f32, tag="rr")

                ET8 = p_et.tile([128, MT, 1024], f8, tag="ET8")
                C1s8 = p_c1.tile([128, MT, 512], f8, tag="C1s8")
                for mt in range(MT):
                    et_ps = ps_et.tile([128, 1024, 2], f8, tag="et")
                    for nt in range(NT):
                        nc.tensor.transpose(
                            et_ps[:, nt * 128:(nt + 1) * 128, 0],
                            E8[:, nt, mt * 128:(mt + 1) * 128],
                            id8,
                        )
                    nc.scalar.copy(ET8[:, mt, 0:512], et_ps[:, 0:512, 0])
                    nc.vector.tensor_copy(ET8[:, mt, 512:1024], et_ps[:, 512:1024, 0])

                    c1_ps = ps_a.tile([128, 512], f32, tag="a")
                    for np_ in range(NT // 2):
                        nc.tensor.matmul(
                            c1_ps,
                            E8[:, 2 * np_:2 * np_ + 2, mt * 128:(mt + 1) * 128],
                            cm[:, 2 * np_:2 * np_ + 2, 0:512],
                            start=(np_ == 0), stop=(np_ == NT // 2 - 1),
                            perf_mode=DR,
                        )
                        nc.tensor.matmul(
                            sm_ps[:, 16 + 2 * mt:18 + 2 * mt],
                            E8[:, 2 * np_:2 * np_ + 2, mt * 128:(mt + 1) * 128],
                            cm[:, 2 * np_:2 * np_ + 2, 512:514],
                            start=(np_ == 0), stop=(np_ == NT // 2 - 1),
                            perf_mode=DR,
                        )
                    nc.vector.reciprocal(
                        rc[:, mt:mt + 1], sm_ps[:, 16 + 2 * mt:17 + 2 * mt]
                    )
                    nc.scalar.mul(
                        rr[:, mt:mt + 1], rc[:, mt:mt + 1],
                        aux_sb[:, b * MT + mt:b * MT + mt + 1],
                    )
                    nc.vector.tensor_scalar(
                        C1s8[:, mt, :], c1_ps, rr[:, mt:mt + 1], None, MUL,
                    )
                return sm_ps, ET8, C1s8

            def out_phase(it, sm_ps, ET8, C1s8, qs, last=False):
                b = it % BL
                # rows first: frees sm_ps ring early for the next batch
                for nt in range(NT):
                    nc.tensor.matmul(
                        sm_ps[:, 2 * nt:2 * nt + 2],
                        ET8[:, :, nt * 128:(nt + 1) * 128],
                        qs[:, :, 512:514],
                        start=True, stop=True, perf_mode=DR,
                    )
                rows16 = p_sm.tile([128, 16], f16, tag="rows")
                nc.vector.tensor_copy(rows16, sm_ps[:, 0:16])
                nc.gpsimd.dma_start(rws_d.ap()[b], rows16)

                out_sb = p_out.tile([128, NT, 1024], f8, tag="out")
                for nt in range(NT):
                    o_ps = ps_o.tile([128, 1024], f32, tag="o")
                    lhsT = ET8[:, :, nt * 128:(nt + 1) * 128]
                    nc.tensor.matmul(
                        o_ps[:, 0:512], lhsT, qs[:, :, 0:512],
                        start=True, stop=True, perf_mode=DR,
                    )
                    nc.tensor.matmul(
                        o_ps[:, 512:1024], lhsT, C1s8[:, :, :],
                        start=True, stop=True, perf_mode=DR,
                    )
                    if last:
                        # split fine across both engines to shrink the tail
                        nc.scalar.copy(out_sb[:, nt, 0:512], o_ps[:, 0:512])
                        nc.vector.tensor_copy(out_sb[:, nt, 512:1024], o_ps[:, 512:1024])
                        nc.gpsimd.dma_start(
                            out_d.ap()[b][:, nt * 1024:(nt + 1) * 1024],
                            out_sb[:, nt, :],
                        )
                    else:
                        if ACT_NT[nt]:
                            nc.scalar.copy(out_sb[:, nt, :], o_ps)
                        else:
                            nc.vector.tensor_copy(out_sb[:, nt, :], o_ps)
                        if nt % 2 == 1:
                            nc.gpsimd.dma_start(
                                out_d.ap()[b][:, (nt - 1) * 1024:(nt + 1) * 1024],
                                out_sb[:, nt - 1:nt + 1, :].rearrange("p a c -> p (a c)"),
                            )

            def out_nt(it, ET8, C1s8, qs, nt, out_sb, last=False):
                b = it % BL
                o_ps = ps_o.tile([128, 1024], f32, tag="o")
                lhsT = ET8[:, :, nt * 128:(nt + 1) * 128]
                nc.tensor.matmul(
                    o_ps[:, 0:512], lhsT, qs[:, :, 0:512],
                    start=True, stop=True, perf_mode=DR,
                )
                nc.tensor.matmul(
                    o_ps[:, 512:1024], lhsT, C1s8[:, :, :],
                    start=True, stop=True, perf_mode=DR,
                )
                if last:
                    nc.scalar.copy(out_sb[:, nt, 0:512], o_ps[:, 0:512])
                    nc.vector.tensor_copy(out_sb[:, nt, 512:1024], o_ps[:, 512:1024])
                    nc.gpsimd.dma_start(
                        out_d.ap()[b][:, nt * 1024:(nt + 1) * 1024],
                        out_sb[:, nt, :],
                    )
                else:
                    if ACT_NT[nt]:
                        nc.scalar.copy(out_sb[:, nt, :], o_ps)
                    else:
                        nc.vector.tensor_copy(out_sb[:, nt, :], o_ps)
                    if nt % 2 == 1:
                        nc.gpsimd.dma_start(
                            out_d.ap()[b][:, (nt - 1) * 1024:(nt + 1) * 1024],
                            out_sb[:, nt - 1:nt + 1, :].rearrange("p a c -> p (a c)"),
                        )

            def rows_part(it, sm_ps, ET8, qs):
                b = it % BL
                for nt in range(NT):
                    nc.tensor.matmul(
                        sm_ps[:, 2 * nt:2 * nt + 2],
                        ET8[:, :, nt * 128:(nt + 1) * 128],
                        qs[:, :, 512:514],
                        start=True, stop=True, perf_mode=DR,
                    )
                rows16 = p_sm.tile([128, 16], f16, tag="rows")
                nc.vector.tensor_copy(rows16, sm_ps[:, 0:16])
                nc.gpsimd.dma_start(rws_d.ap()[b], rows16)

            # software-pipelined emission: S-pairs of batch b interleave with
            # the out-phase of batch b-1; inputs prefetch one period ahead.
            ins = in_phase(0)
            pend = None
            for it in range(n_iters):
                ct, qw, cm, qs = ins
                b = it % BL
                if pend is not None:
                    rows_part(pend[0], pend[1], pend[2], pend[4])
                E8 = p_e.tile([128, NT, 256], f8, tag="E8")
                out_sb = None
                if pend is not None:
                    out_sb = p_out.tile([128, NT, 1024], f8, tag="out")
                for pp in range(NT // 2):
                    s_ps = ps_a.tile([128, 512], f32, tag="a")
                    for j in range(2):
                        nt = 2 * pp + j
                        o = s_ps[:, j * 256:(j + 1) * 256]
                        for dp in range(DC // 2):
                            nc.tensor.matmul(
                                o,
                                ct[:, 2 * dp:2 * dp + 2, nt * 128:(nt + 1) * 128],
                                qw[:, 2 * dp:2 * dp + 2, :],
                                start=(dp == 0), stop=False, perf_mode=DR,
                            )
                        nc.tensor.matmul(
                            o,
                            cw8_sb[:, :, b * 1024 + nt * 128:b * 1024 + (nt + 1) * 128],
                            rex,
                            start=False, stop=True, perf_mode=DR,
                        )
                    nc.scalar.activation(
                        E8[:, 2 * pp:2 * pp + 2, :], s_ps, EXP,
                        bias=0.0, scale=1.0 / SC,
                    )
                    if pend is not None:
                        out_nt(pend[0], pend[2], pend[3], pend[4], 2 * pp, out_sb)
                        out_nt(pend[0], pend[2], pend[3], pend[4], 2 * pp + 1, out_sb)
                if it + 1 < n_iters:
                    ins = in_phase(it + 1)
                sm_ps, ET8, C1s8 = etc1_phase(it, cm, E8)
                pend = (it, sm_ps, ET8, C1s8, qs)

            # drain the final batch
            rows_part(pend[0], pend[1], pend[2], pend[4])
            out_sb = p_out.tile([128, NT, 1024], f8, tag="out")
            for nt in range(NT):
                out_nt(pend[0], pend[2], pend[3], pend[4], nt, out_sb, last=True)

    nc.compile()
    return nc


def get_nc(repeat=1):
    key = ("nc", repeat)
    if key not in _built:
        _built[key] = _build_nc(repeat)
    return _built[key]


def _f8(x):
    return np.ascontiguousarray(x.astype(F8).view(np.uint8))


def _host_prep(context, query, c_mask, q_mask, w):
    context = np.asarray(context, dtype=np.float32)
    query = np.asarray(query, dtype=np.float32)
    c_mask = np.asarray(c_mask)
    q_mask = np.asarray(q_mask)
    w = np.asarray(w, dtype=np.float32).reshape(3 * D)
    wq, wc, wm = w[:D], w[D:2 * D], w[2 * D:]

    czero = c_mask.astype(np.float32)                       # [B, N]
    cwc = context @ wc                                      # [B, N]
    qwq = query @ wq                                        # [B, M]
    expqb4 = np.where(q_mask, np.exp(qwq), 0.0).astype(np.float32) * 0.25

    # [B, N, D] -> [B, 128, NT, D] with n = nt*128 + p
    ctx_p = context.reshape(B, NT, 128, D).transpose(0, 2, 1, 3)
    czero_p = czero.reshape(B, NT, 128).transpose(0, 2, 1)  # [B, 128, NT]
    cwc_p = cwc.reshape(B, NT, 128).transpose(0, 2, 1)

    cm8 = np.zeros((B, 128, NT, 516), dtype=np.uint8)
    cm8[..., 0:512] = _f8(ctx_p * czero_p[..., None])
    cm8[..., 512:514] = _f8(czero_p)[..., None]

    # ctx^T: [B, D, N] -> [B, 128, DC, N] with d = dc*128 + p
    ctxT = context.transpose(0, 2, 1).reshape(B, DC, 128, N).transpose(0, 2, 1, 3)
    ct8 = _f8(ctxT)

    qTwm = (query * (wm * SC)[None, None, :]).transpose(0, 2, 1)
    qw8 = _f8(qTwm.reshape(B, DC, 128, M).transpose(0, 2, 1, 3))

    q_p = query.reshape(B, MT, 128, D).transpose(0, 2, 1, 3)
    eq_p = expqb4.reshape(B, MT, 128).transpose(0, 2, 1)    # [B, 128, MT]
    qs8 = np.zeros((B, 128, MT, 516), dtype=np.uint8)
    qs8[..., 0:512] = _f8(q_p * eq_p[..., None])
    qs8[..., 512:514] = _f8(eq_p)[..., None]

    # cwc a+residual rows: [2, 2, BL*1024] per core, (p, k, b*1024 + n)
    a = (SC * cwc).astype(F8)
    r = (SC * cwc - a.astype(np.float32)).astype(F8)

    in_maps = []
    for c in range(NCORES):
        bs = slice(c * BL, (c + 1) * BL)
        cw8 = np.zeros((2, 2, BL * 1024), dtype=np.uint8)
        cw8[0, 0] = a[bs].reshape(BL * N).view(np.uint8)
        cw8[1, 0] = r[bs].reshape(BL * N).view(np.uint8)
        aux = np.ascontiguousarray(
            eq_p[bs].transpose(1, 0, 2).reshape(128, BL * MT)
        )
        in_maps.append({
            "cm8": np.ascontiguousarray(cm8[bs].reshape(BL, 128, NT * 516)),
            "ct8": np.ascontiguousarray(ct8[bs].reshape(BL, 128, DC * 1024)),
            "qw8": np.ascontiguousarray(qw8[bs].reshape(BL, 128, DC * 256)),
            "qs8": np.ascontiguousarray(qs8[bs].reshape(BL, 128, MT * 516)),
            "cw8": cw8,
            "aux": aux,
        })
    return in_maps


def run_on_device(in_maps, trace=False, repeat=1, **kw):
    from concourse.bass_utils import run_bass_kernel_spmd

    nc = get_nc(repeat)
    return run_bass_kernel_spmd(
        nc, in_maps, core_ids=list(range(NCORES)), trace=trace, **kw
    )


def _assemble(context, results):
    context = np.asarray(context, dtype=np.float32)
    outs, rows = [], []
    for r in results:
        o = np.asarray(r["out"])
        if o.dtype != F8:
            o = o.view(F8)
        outs.append(o.reshape(BL, 128, NT, 1024))
        rows.append(np.asarray(r["rws"]).reshape(BL, 128, 16))
    # [B, 128, NT, 1024] -> [B, N, 1024]
    o = np.concatenate(outs, 0).astype(np.float32)
    o = o.transpose(0, 2, 1, 3).reshape(B, N, 1024)
    rw = np.concatenate(rows, 0).astype(np.float32)[:, :, 0:16:2]
    rw = rw.transpose(0, 2, 1).reshape(B, N)
    inv = 1.0 / rw
    c2q = o[:, :, 0:512] * inv[:, :, None]
    q2c = o[:, :, 512:1024] * inv[:, :, None]
    return np.concatenate(
        [context, c2q, context * c2q, context * q2c], axis=-1
    ).astype(np.float32, copy=False)


def kernel(context, query, c_mask, q_mask, w):
    in_maps = _host_prep(context, query, c_mask, q_mask, w)
    res = run_on_device(in_maps)
    return _assemble(context, res.results)
